# revision 11
# baseline (speedup 1.0000x reference)
"""Trainium2 Bass kernel for 3D multi-head attention (nn_Attention3D).

Problem: x [1, 16, 16, 16, 528] -> full attention over N=4096 tokens,
8 heads of dim 66, qkv + out projections.

Sharding: one head per NeuronCore (8 cores). Each core computes its
head's q/k/v projections, full 4096x4096 attention, and its partial
contribution to the output projection. Host divides each core's
partial by its softmax denominator (carried as an extra output
column), sums the 8 partials and adds the output bias.

v3 pipeline (fp8 DoubleRow AV + dual-engine exp):
  - scores stay bf16 (DoubleRow's disabled fast-weight-load makes fp8
    scores cadence-neutral on HW, so bf16 keeps the accuracy); AV runs
    fp8e4 DoubleRow, pairing two 128-token k-chunks per instruction
    (halves AV instruction count, ~2x AV rate).
  - exp of the 16.7M scores is split across two engines: ScalarE runs
    native Exp into fp8 E tiles on 4-chunk groups (2048-col
    instructions amortize the ~0.5us access/dispatch overhead), DVE
    runs a one-op Schraudolph on 2-chunk pairs: int8(A*s + B) written
    through a bitcast into the fp8 tile -- the int8 grid IS the
    fp8e4m3 exponent/mantissa grid, so the linear-in-log approximation
    lands within fp8 rounding error. ~75/25 ACT/DVE.
  - PSUM: sca [128,4,512] ring-1 (ACT groups + proj pieces), scb
    [128,2,512] ring-1 (DVE pairs), o_ps [80,512] ring-2; the
    deferred-work FIFO (AV, oT cast, out-proj pieces) replays a block
    behind, as in v1.
  - v is quantized to fp8 at the PSUM->SBUF copy into an 80-stride
    layout (DoubleRow k-tile step must be %16==0), with a ones column
    so the softmax denominator rides the AV accumulator row 0.
  - out projection stays bf16; y streams out as bf16 row-half DMAs,
    host divides by the denominator column and sums partials.
Measured rel err vs fp32 reference ~7e-3 (CPU sim of this pipeline).
"""

import numpy as np

import ml_dtypes

BF16_NP = ml_dtypes.bfloat16

EMBED = 528
EOUT = 536  # proj output cols: 528 data + denom col (528) + pad
HD = 66
NHEADS = 8
NT = 4096
NCH = 5  # contraction chunks of 128 (640 = 528 + bias row + pad)

# Schraudolph fast-exp constants: scores arrive pre-scaled by hd^-0.5
# (folded into wq), so A is just 8*log2(e) onto the int8/fp8e4m3 grid.
A_SCH = 8.0 * np.log2(np.e)
B_SCH = 56.0 - 0.35

# per-block group schedule for blocks 1-7: (chunks, engine)
# 'A' = ScalarE Exp, 'D' = DVE Schraudolph. 24 chunks ACT, 8 DVE.
SCHED = [(4, "A"), (2, "D"), (4, "A"), (2, "D"), (4, "A"), (2, "D"),
         (4, "A"), (2, "D"), (4, "A"), (4, "A")]
# block 0 groups (all 4-chunk, one per qk block): alternate A/D
SCHED_B0 = ["A", "D", "A", "D", "A", "D", "A", "D"]


def _build_nc(nt=NT):
    import concourse.tile as tile
    from concourse import bacc, mybir

    F32 = mybir.dt.float32
    BF16 = mybir.dt.bfloat16
    FP8 = mybir.dt.float8e4
    I8 = mybir.dt.int8
    AF = mybir.ActivationFunctionType
    DR = mybir.MatmulPerfMode.DoubleRow
    MULT = mybir.AluOpType.mult
    ADD = mybir.AluOpType.add

    nkc = nt // 128  # k-token chunks (32)
    npair = nkc // 2  # AV pairs per block (16)
    nqb = nt // 512  # q-token blocks (8)

    nc = bacc.Bacc("TRN2", target_bir_lowering=False, debug=False)
    xT_d = nc.dram_tensor("xT", [128, NCH, nt], BF16, kind="ExternalInput").ap()
    wq_d = nc.dram_tensor("wq", [128, NCH, 128], BF16, kind="ExternalInput").ap()
    wk_d = nc.dram_tensor("wk", [128, NCH, 128], BF16, kind="ExternalInput").ap()
    wv_d = nc.dram_tensor("wv", [128, NCH, HD + 2], BF16, kind="ExternalInput").ap()
    wp_d = nc.dram_tensor("wp", [128, EOUT], BF16, kind="ExternalInput").ap()
    y_d = nc.dram_tensor("y", [nt, EOUT], BF16, kind="ExternalOutput").ap()

    with tile.TileContext(nc) as tc:
        with (
            tc.tile_pool(name="const", bufs=1) as constp,
            tc.tile_pool(name="persist", bufs=1) as pp,
            tc.tile_pool(name="ep", bufs=8) as ep,
            tc.tile_pool(name="yp", bufs=4) as yp,
            tc.tile_pool(name="psS", bufs=1, space="PSUM") as psS,
        ):
            wq = constp.tile([128, NCH, 128], BF16, name="wq_sb")
            wk = constp.tile([128, NCH, 128], BF16, name="wk_sb")
            wv = constp.tile([128, NCH, HD + 2], BF16, name="wv_sb")
            wp = constp.tile([128, EOUT], BF16, name="wp_sb")
            warm = constp.tile([128, 16], BF16, name="warm_sb")

            xT = pp.tile([128, NCH, nt], BF16, name="xT_sb")
            # block 0's x in 5 small per-chunk DMAs first (ready ~5us so
            # qk(0) starts early); blocks 1-7 in 10 wide per-chunk DMAs
            for c in range(NCH):
                nc.sync.dma_start(xT[:, c, 0:512], xT_d[:, c, 0:512])
            nc.sync.dma_start(wq[:], wq_d[:])
            nc.sync.dma_start(wk[:], wk_d[:])
            for c in range(NCH):
                nc.sync.dma_start(xT[:, c, 512:2048], xT_d[:, c, 512:2048])
            for c in range(NCH):
                nc.sync.dma_start(xT[:, c, 2048:nt], xT_d[:, c, 2048:nt])
            nc.sync.dma_start(wv[:], wv_d[:])
            nc.sync.dma_start(wp[:], wp_d[:])

            # qT/kT are hd-padded to 128 partitions (rows HD.. stay 0) so
            # scores contract over a full K=128.
            qT = pp.tile([128, nt], BF16, name="qT")
            kT = pp.tile([128, nt], BF16, name="kT")
            # v in fp8 with an 80-byte chunk stride (DoubleRow k-tile dim
            # step must be a multiple of 16); cols 68-79 memset to zero.
            vaug = pp.tile([128, nkc, 80], FP8, name="vaug")
            # out-proj stationary per block, double-buffered; rows 68-127
            # must read zero in the proj matmul, so memset once and only
            # ever write rows 0..67.
            oT = [pp.tile([128, 512], BF16, name=f"oT{i}") for i in range(2)]
            nc.gpsimd.memset(warm[:], 0)
            nc.gpsimd.memset(vaug[:], 0)
            nc.gpsimd.memset(oT[0][:], 0)
            nc.gpsimd.memset(oT[1][:], 0)

            # ---- deferred-work FIFO: AV pairs, oT casts, projections ----
            o_ps_tiles = {}
            avq = []

            def pop_work(budget, floor=0):
                spent = 0
                while len(avq) > floor and spent < budget:
                    item = avq[0]
                    kind = item[0]
                    if kind == "av":
                        _, b, E, loc, j = item
                        if b not in o_ps_tiles:
                            o_ps_tiles[b] = psO.tile(
                                [80, 512], F32, tag="o", bufs=2, name="o_ps"
                            )
                        o_ps = o_ps_tiles[b]
                        nc.tensor.matmul(
                            o_ps[:],
                            vaug[:, 2 * j : 2 * j + 2, :],
                            E[:, 2 * loc : 2 * loc + 2, :],
                            start=(j == 0),
                            stop=(j == npair - 1),
                            perf_mode=DR,
                            skip_group_check=True,
                        )
                        spent += 1
                    elif kind == "cast":
                        b = item[1]
                        nc.vector.tensor_copy(
                            oT[b % 2][0 : HD + 2, :], o_ps_tiles[b][0 : HD + 2, :]
                        )
                    else:  # proj piece: one 128-token slice of block b
                        _, b, t = item
                        oTt = oT[b % 2]
                        pt = psS.tile([128, EOUT], F32, tag="sca", name="pt")
                        r0 = b * 512 + t * 128
                        st = oTt[:, t * 128 : (t + 1) * 128]
                        nc.tensor.matmul(
                            pt[:, :512], st, wp[:, :512], start=True, stop=True
                        )
                        nc.tensor.matmul(
                            pt[:, 512:EOUT],
                            st,
                            wp[:, 512:EOUT],
                            start=True,
                            stop=True,
                        )
                        ysb = yp.tile([128, EOUT], BF16, tag="ysb", name="ysb")
                        nc.vector.tensor_copy(ysb[:], pt[:])
                        # two row-half DMAs land on two queues so the last
                        # writes drain in parallel
                        nc.sync.dma_start(y_d[r0 : r0 + 64, :], ysb[0:64, :])
                        nc.sync.dma_start(
                            y_d[r0 + 64 : r0 + 128, :], ysb[64:128, :]
                        )
                        spent += 1
                    avq.pop(0)

            def push_block_done(b):
                avq.append(("cast", b))
                for t in range(4):
                    avq.append(("proj", b, t))

            def emit_group(b, kc0, gsz, engine):
                qs = slice(b * 512, (b + 1) * 512)
                if gsz == 4:
                    sc = psS.tile([128, 4, 512], F32, tag="sca", name="sca")
                else:
                    sc = psS.tile([128, 2, 512], F32, tag="scb", name="scb")
                for i in range(gsz):
                    kc = kc0 + i
                    nc.tensor.matmul(
                        sc[:, i, :],
                        kT[:, kc * 128 : (kc + 1) * 128],
                        qT[:, qs],
                        start=True,
                        stop=True,
                    )
                E = ep.tile(
                    [128, gsz, 512], FP8, tag=f"E{gsz}", bufs=12, name="E"
                )
                if engine == "D":
                    nc.vector.tensor_scalar(
                        E[:].bitcast(I8), sc[:], A_SCH, B_SCH, MULT, ADD
                    )
                else:
                    nc.scalar.activation(E[:], sc[:], AF.Exp)
                for loc in range(gsz // 2):
                    avq.append(("av", b, E, loc, kc0 // 2 + loc))

            # ---------------- Phase A + block-0 scores ----------------
            with tc.tile_pool(name="psA", bufs=1, space="PSUM") as psA:
                # PE p-state warmup through the sca ring while x DMA lands
                wps = psS.tile([128, 4, 512], F32, tag="sca", name="sca")
                for _ in range(60):
                    nc.tensor.matmul(
                        wps[0:16, 0, 0:16], warm[:], warm[:], start=True, stop=True
                    )

                for b in range(nqb):
                    qs = slice(b * 512, (b + 1) * 512)
                    ps_q = psA.tile([128, 512], F32, tag="qk", bufs=2, name="ps_q")
                    ps_k = psA.tile([128, 512], F32, tag="qk", bufs=2, name="ps_k")
                    for c in range(NCH):
                        for w, ps in ((wq, ps_q), (wk, ps_k)):
                            nc.tensor.matmul(
                                ps[:],
                                w[:, c, :],
                                xT[:, c, qs],
                                start=(c == 0),
                                stop=(c == NCH - 1),
                            )
                    nc.vector.tensor_copy(qT[:, qs], ps_q[:])
                    nc.vector.tensor_copy(kT[:, qs], ps_k[:])
                    # block 0's scores group over the k-chunks this qk block
                    # just produced; exp starts ~9us into the kernel
                    emit_group(0, 4 * b, 4, SCHED_B0[b])
                # block 0's deferred work queues ahead of block 1's early
                # AVs so cast(0) releases the psO ring before they pop
                push_block_done(0)
                # v: paired token-chunk chains; four of block 1's groups
                # interleave so ScalarE/DVE keep working through the v-pass
                early_b1 = {6: 0, 12: 1, 18: 2, 24: 3}
                kc0 = 0
                b1_groups = []
                for gsz, eng in SCHED:
                    b1_groups.append((kc0, gsz, eng))
                    kc0 += gsz
                for t0 in range(0, nkc, 2):
                    psvs = [
                        psA.tile([128, HD + 2], F32, tag="qk", bufs=2, name="ps_v")
                        for _ in range(2)
                    ]
                    for c in range(NCH):
                        for i in range(2):
                            ts_ = slice((t0 + i) * 128, (t0 + i + 1) * 128)
                            nc.tensor.matmul(
                                psvs[i][:],
                                xT[:, c, ts_],
                                wv[:, c, :],
                                start=(c == 0),
                                stop=(c == NCH - 1),
                            )
                    for i in range(2):
                        nc.vector.tensor_copy(
                            vaug[:, t0 + i, 0 : HD + 2], psvs[i][:]
                        )
                    if t0 in early_b1:
                        g0, gsz, eng = b1_groups[early_b1[t0]]
                        emit_group(1, g0, gsz, eng)

            # ------------- Phase B: blocks 1-7 + deferred work -------------
            with tc.tile_pool(name="psO", bufs=1, space="PSUM") as psO:
                for b in range(1, nqb):
                    for gi, (g0, gsz, eng) in enumerate(b1_groups):
                        if b == 1 and gi < 4:
                            continue  # emitted during the v-pass
                        emit_group(b, g0, gsz, eng)
                        pop_work(2 + (gsz == 4), floor=2)
                    push_block_done(b)
                # drain
                pop_work(10**9)

    nc.compile()
    return nc


def _prep_inputs(x, w_qkv, b_qkv, w_proj, nt):
    """Host-side shard prep: returns list of 8 in_maps."""
    x = np.asarray(x, dtype=np.float32)
    w_qkv = np.asarray(w_qkv, dtype=np.float32)
    b_qkv = np.asarray(b_qkv, dtype=np.float32)
    w_proj = np.asarray(w_proj, dtype=np.float32)

    xt = x.reshape(nt, EMBED)
    xT_pad = np.zeros((NCH * 128, nt), dtype=np.float32)
    xT_pad[:EMBED] = xt.T
    xT_pad[EMBED] = 1.0
    # [128, NCH, nt]: partition-major to match the SBUF tile layout
    xT_in = np.ascontiguousarray(
        xT_pad.reshape(NCH, 128, nt).transpose(1, 0, 2)
    ).astype(BF16_NP)

    s = float(HD) ** -0.5
    in_maps = []
    for h in range(NHEADS):
        sl_q = slice(h * HD, (h + 1) * HD)
        sl_k = slice(EMBED + h * HD, EMBED + (h + 1) * HD)
        sl_v = slice(2 * EMBED + h * HD, 2 * EMBED + (h + 1) * HD)

        wq_t = np.zeros((NCH * 128, 128), dtype=np.float32)
        wq_t[:EMBED, :HD] = (w_qkv[sl_q] * s).T
        wq_t[EMBED, :HD] = b_qkv[sl_q] * s

        wk_t = np.zeros((NCH * 128, 128), dtype=np.float32)
        wk_t[:EMBED, :HD] = w_qkv[sl_k].T
        wk_t[EMBED, :HD] = b_qkv[sl_k]

        # ones column at index 0 so the softmax denominator lands on
        # PSUM partition 0 (-> oT row 0)
        wv_t = np.zeros((NCH * 128, HD + 2), dtype=np.float32)
        wv_t[:EMBED, 1 : HD + 1] = w_qkv[sl_v].T
        wv_t[EMBED, 1 : HD + 1] = b_qkv[sl_v]
        wv_t[EMBED, 0] = 1.0

        # proj weights: row 0 = denom row: zero into data cols, 1.0 into
        # col 528 so y[:, 528] = softmax denominator per token
        wp_t = np.zeros((128, EOUT), dtype=np.float32)
        wp_t[1 : HD + 1, :EMBED] = w_proj[:, sl_q].T
        wp_t[0, EMBED] = 1.0

        in_maps.append(
            {
                "xT": xT_in,
                "wq": np.ascontiguousarray(
                    wq_t.reshape(NCH, 128, 128).transpose(1, 0, 2)
                ).astype(BF16_NP),
                "wk": np.ascontiguousarray(
                    wk_t.reshape(NCH, 128, 128).transpose(1, 0, 2)
                ).astype(BF16_NP),
                "wv": np.ascontiguousarray(
                    wv_t.reshape(NCH, 128, HD + 2).transpose(1, 0, 2)
                ).astype(BF16_NP),
                "wp": wp_t.astype(BF16_NP),
            }
        )
    return in_maps


_NC_CACHE = {}


def _get_nc(nt=NT):
    if nt not in _NC_CACHE:
        _NC_CACHE[nt] = _build_nc(nt)
    return _NC_CACHE[nt]


def kernel(x, w_qkv, b_qkv, w_proj, b_proj, _trace=False):
    from concourse.bass_utils import run_bass_kernel_spmd

    x = np.asarray(x, dtype=np.float32)
    b_proj = np.asarray(b_proj, dtype=np.float32)
    B, D, H, W, C = x.shape
    nt = D * H * W

    nc = _get_nc(nt)
    in_maps = _prep_inputs(x, w_qkv, b_qkv, w_proj, nt)
    res = run_bass_kernel_spmd(
        nc, in_maps, core_ids=list(range(NHEADS)), trace=_trace
    )
    out = np.zeros((nt, EMBED), dtype=np.float32)
    for r in res.results:
        yfull = np.asarray(r["y"], dtype=np.float32)
        out += yfull[:, :EMBED] / yfull[:, EMBED : EMBED + 1]
    out += b_proj
    kernel.last_results = res
    return out.reshape(B, D, H, W, C)


# revision 12
# speedup vs baseline: 1.5388x; 1.5388x over previous
"""Trainium2 Bass kernel for 3D multi-head attention (nn_Attention3D).

Problem: x [1, 16, 16, 16, 528] -> full attention over N=4096 tokens,
8 heads of dim 66, qkv + out projections.

Sharding: one head per NeuronCore (8 cores). Each core computes its
head's q/k/v projections, full 4096x4096 attention, and its partial
contribution to the output projection. Host divides each core's
partial by its softmax denominator (carried as an extra output
column), sums the 8 partials and adds the output bias.

v3 pipeline (fp8 DoubleRow AV + dual-engine exp):
  - scores stay bf16 (DoubleRow's disabled fast-weight-load makes fp8
    scores cadence-neutral on HW, so bf16 keeps the accuracy); AV runs
    fp8e4 DoubleRow, pairing two 128-token k-chunks per instruction
    (halves AV instruction count, ~2x AV rate).
  - exp of the 16.7M scores is split across two engines: ScalarE runs
    native Exp into fp8 E tiles on 4-chunk groups (2048-col
    instructions amortize the ~0.5us access/dispatch overhead), DVE
    runs a one-op Schraudolph on 2-chunk pairs: int8(A*s + B) written
    through a bitcast into the fp8 tile -- the int8 grid IS the
    fp8e4m3 exponent/mantissa grid, so the linear-in-log approximation
    lands within fp8 rounding error. ~75/25 ACT/DVE.
  - PSUM: sca [128,4,512] ring-1 (ACT groups + proj pieces), scb
    [128,2,512] ring-1 (DVE pairs), o_ps [80,512] ring-2; the
    deferred-work FIFO (AV, oT cast, out-proj pieces) replays a block
    behind, as in v1.
  - v is quantized to fp8 at the PSUM->SBUF copy into an 80-stride
    layout (DoubleRow k-tile step must be %16==0), with a ones column
    so the softmax denominator rides the AV accumulator row 0.
  - out projection stays bf16; y streams out as bf16 row-half DMAs,
    host divides by the denominator column and sums partials.
Measured rel err vs fp32 reference ~7e-3 (CPU sim of this pipeline).
"""

import numpy as np

import ml_dtypes

BF16_NP = ml_dtypes.bfloat16

EMBED = 528
EOUT = 536  # proj output cols: 528 data + denom col (528) + pad
HD = 66
NHEADS = 8
NT = 4096
NCH = 5  # contraction chunks of 128 (640 = 528 + bias row + pad)

# Schraudolph fast-exp constants: scores arrive pre-scaled by hd^-0.5
# (folded into wq), so A is just 8*log2(e) onto the int8/fp8e4m3 grid.
A_SCH = 8.0 * np.log2(np.e)
B_SCH = 56.0 - 0.35

# exp-engine assignment per score pair (16 pairs of k-chunks per block):
# pairs listed go to DVE (Schraudolph), the rest to ScalarE Exp. DVE
# also carries the y/oT/qkv/v casts, so it gets the smaller share
# (~85/43 pairs overall keeps both engines ~equally busy).
DVE_PAIRS_B0 = frozenset({1, 3, 5, 7, 9, 11, 13, 15})  # 8/16 in block 0
DVE_PAIRS = frozenset({1, 4, 7, 10, 13})  # 5/16 in blocks 1-7


def _build_nc(nt=NT):
    import concourse.tile as tile
    from concourse import bacc, mybir

    F32 = mybir.dt.float32
    BF16 = mybir.dt.bfloat16
    FP8 = mybir.dt.float8e4
    I8 = mybir.dt.int8
    AF = mybir.ActivationFunctionType
    DR = mybir.MatmulPerfMode.DoubleRow
    MULT = mybir.AluOpType.mult
    ADD = mybir.AluOpType.add

    nkc = nt // 128  # k-token chunks (32)
    npair = nkc // 2  # AV pairs per block (16)
    nqb = nt // 512  # q-token blocks (8)

    nc = bacc.Bacc("TRN2", target_bir_lowering=False, debug=False)
    xT_d = nc.dram_tensor("xT", [128, NCH, nt], BF16, kind="ExternalInput").ap()
    wq_d = nc.dram_tensor("wq", [128, NCH, 128], BF16, kind="ExternalInput").ap()
    wk_d = nc.dram_tensor("wk", [128, NCH, 128], BF16, kind="ExternalInput").ap()
    wv_d = nc.dram_tensor("wv", [128, NCH, HD + 2], BF16, kind="ExternalInput").ap()
    wp_d = nc.dram_tensor("wp", [128, EOUT], BF16, kind="ExternalInput").ap()
    y_d = nc.dram_tensor("y", [nt, EOUT], BF16, kind="ExternalOutput").ap()

    with tile.TileContext(nc) as tc:
        with (
            tc.tile_pool(name="const", bufs=1) as constp,
            tc.tile_pool(name="persist", bufs=1) as pp,
            tc.tile_pool(name="ep", bufs=8) as ep,
            tc.tile_pool(name="yp", bufs=4) as yp,
            tc.tile_pool(name="psS", bufs=1, space="PSUM") as psS,
        ):
            wq = constp.tile([128, NCH, 128], BF16, name="wq_sb")
            wk = constp.tile([128, NCH, 128], BF16, name="wk_sb")
            wv = constp.tile([128, NCH, HD + 2], BF16, name="wv_sb")
            wp = constp.tile([128, EOUT], BF16, name="wp_sb")
            warm = constp.tile([128, 16], BF16, name="warm_sb")

            xT = pp.tile([128, NCH, nt], BF16, name="xT_sb")
            # block 0's x in 5 small per-chunk DMAs first (ready ~5us so
            # qk(0) starts early); blocks 1-7 in 10 wide per-chunk DMAs
            for c in range(NCH):
                nc.sync.dma_start(xT[:, c, 0:512], xT_d[:, c, 0:512])
            nc.sync.dma_start(wq[:], wq_d[:])
            nc.sync.dma_start(wk[:], wk_d[:])
            for c in range(NCH):
                nc.sync.dma_start(xT[:, c, 512:2048], xT_d[:, c, 512:2048])
            for c in range(NCH):
                nc.sync.dma_start(xT[:, c, 2048:nt], xT_d[:, c, 2048:nt])
            nc.sync.dma_start(wv[:], wv_d[:])
            nc.sync.dma_start(wp[:], wp_d[:])

            # qT/kT are hd-padded to 128 partitions (rows HD.. stay 0) so
            # scores contract over a full K=128.
            qT = pp.tile([128, nt], BF16, name="qT")
            kT = pp.tile([128, nt], BF16, name="kT")
            # v in fp8 with an 80-byte chunk stride (DoubleRow k-tile dim
            # step must be a multiple of 16); cols 68-79 memset to zero.
            vaug = pp.tile([128, nkc, 80], FP8, name="vaug")
            # out-proj stationary per block, double-buffered; rows 68-127
            # must read zero in the proj matmul, so memset once and only
            # ever write rows 0..67.
            oT = [pp.tile([128, 512], BF16, name=f"oT{i}") for i in range(2)]
            nc.gpsimd.memset(warm[:], 0)
            nc.gpsimd.memset(vaug[:], 0)
            nc.gpsimd.memset(oT[0][:], 0)
            nc.gpsimd.memset(oT[1][:], 0)

            # ---- deferred-work FIFO: AV pairs, oT casts, projections ----
            o_ps_tiles = {}
            avq = []

            def pop_work(budget, floor=0):
                spent = 0
                while len(avq) > floor and spent < budget:
                    item = avq[0]
                    kind = item[0]
                    if kind == "av":
                        _, b, E, loc, j = item
                        if b not in o_ps_tiles:
                            o_ps_tiles[b] = psO.tile(
                                [80, 512], F32, tag="o", bufs=2, name="o_ps"
                            )
                        o_ps = o_ps_tiles[b]
                        nc.tensor.matmul(
                            o_ps[:],
                            vaug[:, 2 * j : 2 * j + 2, :],
                            E[:, 2 * loc : 2 * loc + 2, :],
                            start=(j == 0),
                            stop=(j == npair - 1),
                            perf_mode=DR,
                            skip_group_check=True,
                        )
                        spent += 1
                    elif kind == "cast":
                        b = item[1]
                        nc.vector.tensor_copy(
                            oT[b % 2][0 : HD + 2, :], o_ps_tiles[b][0 : HD + 2, :]
                        )
                    else:  # proj piece: one 128-token slice of block b
                        _, b, t = item
                        oTt = oT[b % 2]
                        pt = psS.tile([128, EOUT], F32, tag="sc", bufs=3, name="pt")
                        r0 = b * 512 + t * 128
                        st = oTt[:, t * 128 : (t + 1) * 128]
                        nc.tensor.matmul(
                            pt[:, :512], st, wp[:, :512], start=True, stop=True
                        )
                        nc.tensor.matmul(
                            pt[:, 512:EOUT],
                            st,
                            wp[:, 512:EOUT],
                            start=True,
                            stop=True,
                        )
                        ysb = yp.tile([128, EOUT], BF16, tag="ysb", name="ysb")
                        nc.vector.tensor_copy(ysb[:], pt[:])
                        # two row-half DMAs land on two queues so the last
                        # writes drain in parallel
                        nc.sync.dma_start(y_d[r0 : r0 + 64, :], ysb[0:64, :])
                        nc.sync.dma_start(
                            y_d[r0 + 64 : r0 + 128, :], ysb[64:128, :]
                        )
                        spent += 1
                    avq.pop(0)

            def push_block_done(b):
                avq.append(("cast", b))
                for t in range(4):
                    avq.append(("proj", b, t))

            def emit_pair(b, j):
                qs = slice(b * 512, (b + 1) * 512)
                sc = psS.tile([128, 2, 512], F32, tag="sc", bufs=3, name="sc")
                for i in range(2):
                    kc = 2 * j + i
                    nc.tensor.matmul(
                        sc[:, i, :],
                        kT[:, kc * 128 : (kc + 1) * 128],
                        qT[:, qs],
                        start=True,
                        stop=True,
                    )
                E = ep.tile([128, 2, 512], FP8, tag="E", bufs=16, name="E")
                use_dve = j in (DVE_PAIRS_B0 if b == 0 else DVE_PAIRS)
                if use_dve:
                    nc.vector.tensor_scalar(
                        E[:].bitcast(I8), sc[:], A_SCH, B_SCH, MULT, ADD
                    )
                else:
                    nc.scalar.activation(E[:], sc[:], AF.Exp)
                avq.append(("av", b, E, 0, j))

            # ---------------- Phase A + block-0 scores ----------------
            with tc.tile_pool(name="psA", bufs=1, space="PSUM") as psA:
                # PE p-state warmup through the sca ring while x DMA lands
                wps = psS.tile([128, 2, 512], F32, tag="sc", bufs=3, name="sc")
                for _ in range(60):
                    nc.tensor.matmul(
                        wps[0:16, 0, 0:16], warm[:], warm[:], start=True, stop=True
                    )

                for b in range(nqb):
                    qs = slice(b * 512, (b + 1) * 512)
                    ps_q = psA.tile([128, 512], F32, tag="qk", bufs=2, name="ps_q")
                    ps_k = psA.tile([128, 512], F32, tag="qk", bufs=2, name="ps_k")
                    for c in range(NCH):
                        for w, ps in ((wq, ps_q), (wk, ps_k)):
                            nc.tensor.matmul(
                                ps[:],
                                w[:, c, :],
                                xT[:, c, qs],
                                start=(c == 0),
                                stop=(c == NCH - 1),
                            )
                    nc.vector.tensor_copy(qT[:, qs], ps_q[:])
                    nc.vector.tensor_copy(kT[:, qs], ps_k[:])
                    # block 0's score pairs over the k-chunks this qk block
                    # just produced; exp starts ~9us into the kernel
                    emit_pair(0, 2 * b)
                    emit_pair(0, 2 * b + 1)
                # block 0's deferred work queues ahead of block 1's early
                # AVs so cast(0) releases the psO ring before they pop
                push_block_done(0)
                # v: two token-block chains in flight; six of block 1's
                # score pairs interleave so ScalarE/DVE keep working
                # through the v-pass
                early_b1 = {6: 0, 10: 1, 14: 2, 18: 3, 22: 4, 26: 5}
                for t0 in range(0, nkc, 2):
                    psvs = [
                        psA.tile([128, HD + 2], F32, tag="qk", bufs=2, name="ps_v")
                        for _ in range(2)
                    ]
                    for c in range(NCH):
                        for i in range(2):
                            ts_ = slice((t0 + i) * 128, (t0 + i + 1) * 128)
                            nc.tensor.matmul(
                                psvs[i][:],
                                xT[:, c, ts_],
                                wv[:, c, :],
                                start=(c == 0),
                                stop=(c == NCH - 1),
                            )
                    for i in range(2):
                        nc.vector.tensor_copy(
                            vaug[:, t0 + i, 0 : HD + 2], psvs[i][:]
                        )
                    if t0 in early_b1:
                        emit_pair(1, early_b1[t0])

            # ------------- Phase B: blocks 1-7 + deferred work -------------
            with tc.tile_pool(name="psO", bufs=1, space="PSUM") as psO:
                for b in range(1, nqb):
                    for j in range(npair):
                        if b == 1 and j < 6:
                            continue  # emitted during the v-pass
                        emit_pair(b, j)
                        pop_work(2, floor=2)
                    push_block_done(b)
                # drain
                pop_work(10**9)

    nc.compile()
    return nc


def _prep_inputs(x, w_qkv, b_qkv, w_proj, nt):
    """Host-side shard prep: returns list of 8 in_maps."""
    x = np.asarray(x, dtype=np.float32)
    w_qkv = np.asarray(w_qkv, dtype=np.float32)
    b_qkv = np.asarray(b_qkv, dtype=np.float32)
    w_proj = np.asarray(w_proj, dtype=np.float32)

    xt = x.reshape(nt, EMBED)
    xT_pad = np.zeros((NCH * 128, nt), dtype=np.float32)
    xT_pad[:EMBED] = xt.T
    xT_pad[EMBED] = 1.0
    # [128, NCH, nt]: partition-major to match the SBUF tile layout
    xT_in = np.ascontiguousarray(
        xT_pad.reshape(NCH, 128, nt).transpose(1, 0, 2)
    ).astype(BF16_NP)

    s = float(HD) ** -0.5
    in_maps = []
    for h in range(NHEADS):
        sl_q = slice(h * HD, (h + 1) * HD)
        sl_k = slice(EMBED + h * HD, EMBED + (h + 1) * HD)
        sl_v = slice(2 * EMBED + h * HD, 2 * EMBED + (h + 1) * HD)

        wq_t = np.zeros((NCH * 128, 128), dtype=np.float32)
        wq_t[:EMBED, :HD] = (w_qkv[sl_q] * s).T
        wq_t[EMBED, :HD] = b_qkv[sl_q] * s

        wk_t = np.zeros((NCH * 128, 128), dtype=np.float32)
        wk_t[:EMBED, :HD] = w_qkv[sl_k].T
        wk_t[EMBED, :HD] = b_qkv[sl_k]

        # ones column at index 0 so the softmax denominator lands on
        # PSUM partition 0 (-> oT row 0)
        wv_t = np.zeros((NCH * 128, HD + 2), dtype=np.float32)
        wv_t[:EMBED, 1 : HD + 1] = w_qkv[sl_v].T
        wv_t[EMBED, 1 : HD + 1] = b_qkv[sl_v]
        wv_t[EMBED, 0] = 1.0

        # proj weights: row 0 = denom row: zero into data cols, 1.0 into
        # col 528 so y[:, 528] = softmax denominator per token
        wp_t = np.zeros((128, EOUT), dtype=np.float32)
        wp_t[1 : HD + 1, :EMBED] = w_proj[:, sl_q].T
        wp_t[0, EMBED] = 1.0

        in_maps.append(
            {
                "xT": xT_in,
                "wq": np.ascontiguousarray(
                    wq_t.reshape(NCH, 128, 128).transpose(1, 0, 2)
                ).astype(BF16_NP),
                "wk": np.ascontiguousarray(
                    wk_t.reshape(NCH, 128, 128).transpose(1, 0, 2)
                ).astype(BF16_NP),
                "wv": np.ascontiguousarray(
                    wv_t.reshape(NCH, 128, HD + 2).transpose(1, 0, 2)
                ).astype(BF16_NP),
                "wp": wp_t.astype(BF16_NP),
            }
        )
    return in_maps


_NC_CACHE = {}


def _get_nc(nt=NT):
    if nt not in _NC_CACHE:
        _NC_CACHE[nt] = _build_nc(nt)
    return _NC_CACHE[nt]


def kernel(x, w_qkv, b_qkv, w_proj, b_proj, _trace=False):
    from concourse.bass_utils import run_bass_kernel_spmd

    x = np.asarray(x, dtype=np.float32)
    b_proj = np.asarray(b_proj, dtype=np.float32)
    B, D, H, W, C = x.shape
    nt = D * H * W

    nc = _get_nc(nt)
    in_maps = _prep_inputs(x, w_qkv, b_qkv, w_proj, nt)
    res = run_bass_kernel_spmd(
        nc, in_maps, core_ids=list(range(NHEADS)), trace=_trace
    )
    out = np.zeros((nt, EMBED), dtype=np.float32)
    for r in res.results:
        yfull = np.asarray(r["y"], dtype=np.float32)
        out += yfull[:, :EMBED] / yfull[:, EMBED : EMBED + 1]
    out += b_proj
    kernel.last_results = res
    return out.reshape(B, D, H, W, C)


# revision 13
# speedup vs baseline: 1.5498x; 1.0071x over previous
"""Trainium2 Bass kernel for 3D multi-head attention (nn_Attention3D).

Problem: x [1, 16, 16, 16, 528] -> full attention over N=4096 tokens,
8 heads of dim 66, qkv + out projections.

Sharding: one head per NeuronCore (8 cores). Each core computes its
head's q/k/v projections, full 4096x4096 attention, and its partial
contribution to the output projection. Host divides each core's
partial by its softmax denominator (carried as an extra output
column), sums the 8 partials and adds the output bias.

v3 pipeline (fp8 DoubleRow AV + dual-engine exp):
  - scores stay bf16 (DoubleRow's disabled fast-weight-load makes fp8
    scores cadence-neutral on HW, so bf16 keeps the accuracy); AV runs
    fp8e4 DoubleRow, pairing two 128-token k-chunks per instruction
    (halves AV instruction count, ~2x AV rate).
  - exp of the 16.7M scores is split across two engines: ScalarE runs
    native Exp into fp8 E tiles on 4-chunk groups (2048-col
    instructions amortize the ~0.5us access/dispatch overhead), DVE
    runs a one-op Schraudolph on 2-chunk pairs: int8(A*s + B) written
    through a bitcast into the fp8 tile -- the int8 grid IS the
    fp8e4m3 exponent/mantissa grid, so the linear-in-log approximation
    lands within fp8 rounding error. ~75/25 ACT/DVE.
  - PSUM: sca [128,4,512] ring-1 (ACT groups + proj pieces), scb
    [128,2,512] ring-1 (DVE pairs), o_ps [80,512] ring-2; the
    deferred-work FIFO (AV, oT cast, out-proj pieces) replays a block
    behind, as in v1.
  - v is quantized to fp8 at the PSUM->SBUF copy into an 80-stride
    layout (DoubleRow k-tile step must be %16==0), with a ones column
    so the softmax denominator rides the AV accumulator row 0.
  - out projection stays bf16; y streams out as bf16 row-half DMAs,
    host divides by the denominator column and sums partials.
Measured rel err vs fp32 reference ~7e-3 (CPU sim of this pipeline).
"""

import numpy as np

import ml_dtypes

BF16_NP = ml_dtypes.bfloat16

EMBED = 528
EOUT = 536  # proj output cols: 528 data + denom col (528) + pad
HD = 66
NHEADS = 8
NT = 4096
NCH = 5  # contraction chunks of 128 (640 = 528 + bias row + pad)

# Schraudolph fast-exp constants: scores arrive pre-scaled by hd^-0.5
# (folded into wq), so A is just 8*log2(e) onto the int8/fp8e4m3 grid.
A_SCH = 8.0 * np.log2(np.e)
B_SCH = 56.0 - 0.35

# exp-engine assignment per score pair (16 pairs of k-chunks per block):
# pairs listed go to DVE (Schraudolph), the rest to ScalarE Exp. DVE
# also carries the y/oT/qkv/v casts, so it gets the smaller share
# (~85/43 pairs overall keeps both engines ~equally busy).
DVE_PAIRS_B0 = frozenset({1, 3, 5, 7, 9, 11, 13, 15})  # 8/16 in block 0
DVE_PAIRS = frozenset({1, 4, 7, 10, 12, 15})  # 6/16 in blocks 1-7


def _build_nc(nt=NT):
    import concourse.tile as tile
    from concourse import bacc, mybir

    F32 = mybir.dt.float32
    BF16 = mybir.dt.bfloat16
    FP8 = mybir.dt.float8e4
    I8 = mybir.dt.int8
    AF = mybir.ActivationFunctionType
    DR = mybir.MatmulPerfMode.DoubleRow
    MULT = mybir.AluOpType.mult
    ADD = mybir.AluOpType.add

    nkc = nt // 128  # k-token chunks (32)
    npair = nkc // 2  # AV pairs per block (16)
    nqb = nt // 512  # q-token blocks (8)

    nc = bacc.Bacc("TRN2", target_bir_lowering=False, debug=False)
    xT_d = nc.dram_tensor("xT", [128, NCH, nt], BF16, kind="ExternalInput").ap()
    wq_d = nc.dram_tensor("wq", [128, NCH, 128], BF16, kind="ExternalInput").ap()
    wk_d = nc.dram_tensor("wk", [128, NCH, 128], BF16, kind="ExternalInput").ap()
    wv_d = nc.dram_tensor("wv", [128, NCH, HD + 2], BF16, kind="ExternalInput").ap()
    wp_d = nc.dram_tensor("wp", [128, EOUT], BF16, kind="ExternalInput").ap()
    y_d = nc.dram_tensor("y", [nt, EOUT], BF16, kind="ExternalOutput").ap()

    with tile.TileContext(nc) as tc:
        with (
            tc.tile_pool(name="const", bufs=1) as constp,
            tc.tile_pool(name="persist", bufs=1) as pp,
            tc.tile_pool(name="ep", bufs=8) as ep,
            tc.tile_pool(name="yp", bufs=4) as yp,
            tc.tile_pool(name="psS", bufs=1, space="PSUM") as psS,
        ):
            wq = constp.tile([128, NCH, 128], BF16, name="wq_sb")
            wk = constp.tile([128, NCH, 128], BF16, name="wk_sb")
            wv = constp.tile([128, NCH, HD + 2], BF16, name="wv_sb")
            wp = constp.tile([128, EOUT], BF16, name="wp_sb")
            warm = constp.tile([128, 16], BF16, name="warm_sb")

            xT = pp.tile([128, NCH, nt], BF16, name="xT_sb")
            # block 0's x + qk weights first (one strided DMA each, so
            # qk(0) starts ~7us in); the rest of x in two wide DMAs that
            # land on other queues
            nc.sync.dma_start(xT[:, :, 0:512], xT_d[:, :, 0:512])
            nc.sync.dma_start(wq[:], wq_d[:])
            nc.sync.dma_start(wk[:], wk_d[:])
            nc.sync.dma_start(xT[:, :, 512:2048], xT_d[:, :, 512:2048])
            nc.sync.dma_start(xT[:, :, 2048:nt], xT_d[:, :, 2048:nt])
            nc.sync.dma_start(wv[:], wv_d[:])
            nc.sync.dma_start(wp[:], wp_d[:])

            # qT/kT are hd-padded to 128 partitions (rows HD.. stay 0) so
            # scores contract over a full K=128.
            qT = pp.tile([128, nt], BF16, name="qT")
            kT = pp.tile([128, nt], BF16, name="kT")
            # v in fp8 with an 80-byte chunk stride (DoubleRow k-tile dim
            # step must be a multiple of 16); cols 68-79 memset to zero.
            vaug = pp.tile([128, nkc, 80], FP8, name="vaug")
            # out-proj stationary per block, double-buffered; rows 68-127
            # must read zero in the proj matmul, so memset once and only
            # ever write rows 0..67.
            oT = [pp.tile([128, 512], BF16, name=f"oT{i}") for i in range(2)]
            nc.gpsimd.memset(warm[:], 0)
            nc.gpsimd.memset(vaug[:], 0)
            nc.gpsimd.memset(oT[0][:], 0)
            nc.gpsimd.memset(oT[1][:], 0)

            # ---- deferred-work FIFO: AV pairs, oT casts, projections ----
            o_ps_tiles = {}
            avq = []

            def pop_work(budget, floor=0):
                spent = 0
                while len(avq) > floor and spent < budget:
                    item = avq[0]
                    kind = item[0]
                    if kind == "av":
                        _, b, E, loc, j = item
                        if b not in o_ps_tiles:
                            o_ps_tiles[b] = psO.tile(
                                [80, 512], F32, tag="o", bufs=2, name="o_ps"
                            )
                        o_ps = o_ps_tiles[b]
                        nc.tensor.matmul(
                            o_ps[:],
                            vaug[:, 2 * j : 2 * j + 2, :],
                            E[:, 2 * loc : 2 * loc + 2, :],
                            start=(j == 0),
                            stop=(j == npair - 1),
                            perf_mode=DR,
                            skip_group_check=True,
                        )
                        spent += 1
                    elif kind == "cast":
                        b = item[1]
                        nc.vector.tensor_copy(
                            oT[b % 2][0 : HD + 2, :], o_ps_tiles[b][0 : HD + 2, :]
                        )
                    else:  # proj piece: one 128-token slice of block b
                        _, b, t = item
                        oTt = oT[b % 2]
                        pt = psS.tile([128, EOUT], F32, tag="sc", bufs=3, name="pt")
                        r0 = b * 512 + t * 128
                        st = oTt[:, t * 128 : (t + 1) * 128]
                        nc.tensor.matmul(
                            pt[:, :512], st, wp[:, :512], start=True, stop=True
                        )
                        nc.tensor.matmul(
                            pt[:, 512:EOUT],
                            st,
                            wp[:, 512:EOUT],
                            start=True,
                            stop=True,
                        )
                        ysb = yp.tile([128, EOUT], BF16, tag="ysb", name="ysb")
                        nc.vector.tensor_copy(ysb[:], pt[:])
                        # two row-half DMAs land on two queues so the last
                        # writes drain in parallel
                        nc.sync.dma_start(y_d[r0 : r0 + 64, :], ysb[0:64, :])
                        nc.sync.dma_start(
                            y_d[r0 + 64 : r0 + 128, :], ysb[64:128, :]
                        )
                        spent += 1
                    avq.pop(0)

            def push_block_done(b):
                avq.append(("cast", b))
                for t in range(4):
                    avq.append(("proj", b, t))

            def emit_pair(b, j):
                qs = slice(b * 512, (b + 1) * 512)
                sc = psS.tile([128, 2, 512], F32, tag="sc", bufs=3, name="sc")
                for i in range(2):
                    kc = 2 * j + i
                    nc.tensor.matmul(
                        sc[:, i, :],
                        kT[:, kc * 128 : (kc + 1) * 128],
                        qT[:, qs],
                        start=True,
                        stop=True,
                    )
                E = ep.tile([128, 2, 512], FP8, tag="E", bufs=16, name="E")
                use_dve = j in (DVE_PAIRS_B0 if b == 0 else DVE_PAIRS)
                if use_dve:
                    nc.vector.tensor_scalar(
                        E[:].bitcast(I8), sc[:], A_SCH, B_SCH, MULT, ADD
                    )
                else:
                    nc.scalar.activation(E[:], sc[:], AF.Exp)
                avq.append(("av", b, E, 0, j))

            # ---------------- Phase A + block-0 scores ----------------
            with tc.tile_pool(name="psA", bufs=1, space="PSUM") as psA:
                # PE p-state warmup through the sca ring while x DMA lands
                wps = psS.tile([128, 2, 512], F32, tag="sc", bufs=3, name="sc")
                for _ in range(24):
                    nc.tensor.matmul(
                        wps[0:16, 0, 0:16], warm[:], warm[:], start=True, stop=True
                    )

                for b in range(nqb):
                    qs = slice(b * 512, (b + 1) * 512)
                    ps_q = psA.tile([128, 512], F32, tag="qk", bufs=2, name="ps_q")
                    ps_k = psA.tile([128, 512], F32, tag="qk", bufs=2, name="ps_k")
                    for c in range(NCH):
                        for w, ps in ((wq, ps_q), (wk, ps_k)):
                            nc.tensor.matmul(
                                ps[:],
                                w[:, c, :],
                                xT[:, c, qs],
                                start=(c == 0),
                                stop=(c == NCH - 1),
                            )
                    nc.vector.tensor_copy(qT[:, qs], ps_q[:])
                    nc.vector.tensor_copy(kT[:, qs], ps_k[:])
                    # block 0's score pairs over the k-chunks this qk block
                    # just produced; exp starts ~9us into the kernel
                    emit_pair(0, 2 * b)
                    emit_pair(0, 2 * b + 1)
                # block 0's deferred work queues ahead of block 1's early
                # AVs so cast(0) releases the psO ring before they pop
                push_block_done(0)
                # v: two token-block chains in flight; six of block 1's
                # score pairs interleave so ScalarE/DVE keep working
                # through the v-pass
                early_b1 = {6: 0, 10: 1, 14: 2, 18: 3, 22: 4, 26: 5}
                for t0 in range(0, nkc, 2):
                    psvs = [
                        psA.tile([128, HD + 2], F32, tag="qk", bufs=2, name="ps_v")
                        for _ in range(2)
                    ]
                    for c in range(NCH):
                        for i in range(2):
                            ts_ = slice((t0 + i) * 128, (t0 + i + 1) * 128)
                            nc.tensor.matmul(
                                psvs[i][:],
                                xT[:, c, ts_],
                                wv[:, c, :],
                                start=(c == 0),
                                stop=(c == NCH - 1),
                            )
                    for i in range(2):
                        nc.vector.tensor_copy(
                            vaug[:, t0 + i, 0 : HD + 2], psvs[i][:]
                        )
                    if t0 in early_b1:
                        emit_pair(1, early_b1[t0])

            # ------------- Phase B: blocks 1-7 + deferred work -------------
            with tc.tile_pool(name="psO", bufs=1, space="PSUM") as psO:
                for b in range(1, nqb):
                    last = b == nqb - 1
                    for j in range(npair):
                        if b == 1 and j < 6:
                            continue  # emitted during the v-pass
                        emit_pair(b, j)
                        pop_work(3 if last else 2, floor=0 if last else 2)
                    push_block_done(b)
                # drain
                pop_work(10**9)

    nc.compile()
    return nc


def _prep_inputs(x, w_qkv, b_qkv, w_proj, nt):
    """Host-side shard prep: returns list of 8 in_maps."""
    x = np.asarray(x, dtype=np.float32)
    w_qkv = np.asarray(w_qkv, dtype=np.float32)
    b_qkv = np.asarray(b_qkv, dtype=np.float32)
    w_proj = np.asarray(w_proj, dtype=np.float32)

    xt = x.reshape(nt, EMBED)
    xT_pad = np.zeros((NCH * 128, nt), dtype=np.float32)
    xT_pad[:EMBED] = xt.T
    xT_pad[EMBED] = 1.0
    # [128, NCH, nt]: partition-major to match the SBUF tile layout
    xT_in = np.ascontiguousarray(
        xT_pad.reshape(NCH, 128, nt).transpose(1, 0, 2)
    ).astype(BF16_NP)

    s = float(HD) ** -0.5
    in_maps = []
    for h in range(NHEADS):
        sl_q = slice(h * HD, (h + 1) * HD)
        sl_k = slice(EMBED + h * HD, EMBED + (h + 1) * HD)
        sl_v = slice(2 * EMBED + h * HD, 2 * EMBED + (h + 1) * HD)

        wq_t = np.zeros((NCH * 128, 128), dtype=np.float32)
        wq_t[:EMBED, :HD] = (w_qkv[sl_q] * s).T
        wq_t[EMBED, :HD] = b_qkv[sl_q] * s

        wk_t = np.zeros((NCH * 128, 128), dtype=np.float32)
        wk_t[:EMBED, :HD] = w_qkv[sl_k].T
        wk_t[EMBED, :HD] = b_qkv[sl_k]

        # ones column at index 0 so the softmax denominator lands on
        # PSUM partition 0 (-> oT row 0)
        wv_t = np.zeros((NCH * 128, HD + 2), dtype=np.float32)
        wv_t[:EMBED, 1 : HD + 1] = w_qkv[sl_v].T
        wv_t[EMBED, 1 : HD + 1] = b_qkv[sl_v]
        wv_t[EMBED, 0] = 1.0

        # proj weights: row 0 = denom row: zero into data cols, 1.0 into
        # col 528 so y[:, 528] = softmax denominator per token
        wp_t = np.zeros((128, EOUT), dtype=np.float32)
        wp_t[1 : HD + 1, :EMBED] = w_proj[:, sl_q].T
        wp_t[0, EMBED] = 1.0

        in_maps.append(
            {
                "xT": xT_in,
                "wq": np.ascontiguousarray(
                    wq_t.reshape(NCH, 128, 128).transpose(1, 0, 2)
                ).astype(BF16_NP),
                "wk": np.ascontiguousarray(
                    wk_t.reshape(NCH, 128, 128).transpose(1, 0, 2)
                ).astype(BF16_NP),
                "wv": np.ascontiguousarray(
                    wv_t.reshape(NCH, 128, HD + 2).transpose(1, 0, 2)
                ).astype(BF16_NP),
                "wp": wp_t.astype(BF16_NP),
            }
        )
    return in_maps


_NC_CACHE = {}


def _get_nc(nt=NT):
    if nt not in _NC_CACHE:
        _NC_CACHE[nt] = _build_nc(nt)
    return _NC_CACHE[nt]


def kernel(x, w_qkv, b_qkv, w_proj, b_proj, _trace=False):
    from concourse.bass_utils import run_bass_kernel_spmd

    x = np.asarray(x, dtype=np.float32)
    b_proj = np.asarray(b_proj, dtype=np.float32)
    B, D, H, W, C = x.shape
    nt = D * H * W

    nc = _get_nc(nt)
    in_maps = _prep_inputs(x, w_qkv, b_qkv, w_proj, nt)
    res = run_bass_kernel_spmd(
        nc, in_maps, core_ids=list(range(NHEADS)), trace=_trace
    )
    out = np.zeros((nt, EMBED), dtype=np.float32)
    for r in res.results:
        yfull = np.asarray(r["y"], dtype=np.float32)
        out += yfull[:, :EMBED] / yfull[:, EMBED : EMBED + 1]
    out += b_proj
    kernel.last_results = res
    return out.reshape(B, D, H, W, C)


# revision 14
# speedup vs baseline: 1.5559x; 1.0040x over previous
"""Trainium2 Bass kernel for 3D multi-head attention (nn_Attention3D).

Problem: x [1, 16, 16, 16, 528] -> full attention over N=4096 tokens,
8 heads of dim 66, qkv + out projections.

Sharding: one head per NeuronCore (8 cores). Each core computes its
head's q/k/v projections, full 4096x4096 attention, and its partial
contribution to the output projection. Host divides each core's
partial by its softmax denominator (carried as an extra output
column), sums the 8 partials and adds the output bias.

v3 pipeline (fp8 DoubleRow AV + dual-engine exp):
  - scores stay bf16 (DoubleRow's disabled fast-weight-load makes fp8
    scores cadence-neutral on HW, so bf16 keeps the accuracy); AV runs
    fp8e4 DoubleRow, pairing two 128-token k-chunks per instruction
    (halves AV instruction count, ~2x AV rate).
  - exp of the 16.7M scores is split across two engines: ScalarE runs
    native Exp into fp8 E tiles on 4-chunk groups (2048-col
    instructions amortize the ~0.5us access/dispatch overhead), DVE
    runs a one-op Schraudolph on 2-chunk pairs: int8(A*s + B) written
    through a bitcast into the fp8 tile -- the int8 grid IS the
    fp8e4m3 exponent/mantissa grid, so the linear-in-log approximation
    lands within fp8 rounding error. ~75/25 ACT/DVE.
  - PSUM: sca [128,4,512] ring-1 (ACT groups + proj pieces), scb
    [128,2,512] ring-1 (DVE pairs), o_ps [80,512] ring-2; the
    deferred-work FIFO (AV, oT cast, out-proj pieces) replays a block
    behind, as in v1.
  - v is quantized to fp8 at the PSUM->SBUF copy into an 80-stride
    layout (DoubleRow k-tile step must be %16==0), with a ones column
    so the softmax denominator rides the AV accumulator row 0.
  - out projection stays bf16; y streams out as bf16 row-half DMAs,
    host divides by the denominator column and sums partials.
Measured rel err vs fp32 reference ~7e-3 (CPU sim of this pipeline).
"""

import numpy as np

import ml_dtypes

BF16_NP = ml_dtypes.bfloat16

EMBED = 528
EOUT = 536  # proj output cols: 528 data + denom col (528) + pad
HD = 66
NHEADS = 8
NT = 4096
NCH = 5  # contraction chunks of 128 (640 = 528 + bias row + pad)

# Schraudolph fast-exp constants: scores arrive pre-scaled by hd^-0.5
# (folded into wq), so A is just 8*log2(e) onto the int8/fp8e4m3 grid.
A_SCH = 8.0 * np.log2(np.e)
B_SCH = 56.0 - 0.35

# exp-engine assignment per score pair (16 pairs of k-chunks per block):
# pairs listed go to DVE (Schraudolph), the rest to ScalarE Exp. DVE
# also carries the y/oT/qkv/v casts, so it gets the smaller share
# (~85/43 pairs overall keeps both engines ~equally busy).
DVE_PAIRS_B0 = frozenset({1, 3, 5, 7, 9, 11, 13, 15})  # 8/16 in block 0
DVE_PAIRS = frozenset({1, 4, 7, 10, 12, 15})  # 6/16 in blocks 1-7


def _build_nc(nt=NT):
    import concourse.tile as tile
    from concourse import bacc, mybir

    F32 = mybir.dt.float32
    BF16 = mybir.dt.bfloat16
    FP8 = mybir.dt.float8e4
    I8 = mybir.dt.int8
    AF = mybir.ActivationFunctionType
    DR = mybir.MatmulPerfMode.DoubleRow
    MULT = mybir.AluOpType.mult
    ADD = mybir.AluOpType.add

    nkc = nt // 128  # k-token chunks (32)
    npair = nkc // 2  # AV pairs per block (16)
    nqb = nt // 512  # q-token blocks (8)

    nc = bacc.Bacc("TRN2", target_bir_lowering=False, debug=False)
    xT_d = nc.dram_tensor("xT", [128, NCH, nt], BF16, kind="ExternalInput").ap()
    wq_d = nc.dram_tensor("wq", [128, NCH, 128], BF16, kind="ExternalInput").ap()
    wk_d = nc.dram_tensor("wk", [128, NCH, 128], BF16, kind="ExternalInput").ap()
    wv_d = nc.dram_tensor("wv", [128, NCH, HD + 2], BF16, kind="ExternalInput").ap()
    wp_d = nc.dram_tensor("wp", [128, EOUT], BF16, kind="ExternalInput").ap()
    y_d = nc.dram_tensor("y", [nt // 512, 128, 4, EOUT], BF16, kind="ExternalOutput").ap()

    with tile.TileContext(nc) as tc:
        with (
            tc.tile_pool(name="const", bufs=1) as constp,
            tc.tile_pool(name="persist", bufs=1) as pp,
            tc.tile_pool(name="ep", bufs=8) as ep,
            tc.tile_pool(name="yp", bufs=4) as yp,
            tc.tile_pool(name="psS", bufs=1, space="PSUM") as psS,
        ):
            wq = constp.tile([128, NCH, 128], BF16, name="wq_sb")
            wk = constp.tile([128, NCH, 128], BF16, name="wk_sb")
            wv = constp.tile([128, NCH, HD + 2], BF16, name="wv_sb")
            wp = constp.tile([128, EOUT], BF16, name="wp_sb")
            warm = constp.tile([128, 16], BF16, name="warm_sb")

            xT = pp.tile([128, NCH, nt], BF16, name="xT_sb")
            # block 0's x + qk weights first (one strided DMA each, so
            # qk(0) starts ~7us in); the rest of x in two wide DMAs that
            # land on other queues
            nc.sync.dma_start(xT[:, 0:2, 0:512], xT_d[:, 0:2, 0:512])
            nc.sync.dma_start(wq[:], wq_d[:])
            nc.sync.dma_start(xT[:, 2:NCH, 0:512], xT_d[:, 2:NCH, 0:512])
            nc.sync.dma_start(wk[:], wk_d[:])
            nc.sync.dma_start(xT[:, :, 512:2048], xT_d[:, :, 512:2048])
            nc.sync.dma_start(xT[:, :, 2048:nt], xT_d[:, :, 2048:nt])
            nc.sync.dma_start(wv[:], wv_d[:])
            nc.sync.dma_start(wp[:], wp_d[:])

            # qT/kT are hd-padded to 128 partitions (rows HD.. stay 0) so
            # scores contract over a full K=128.
            qT = pp.tile([128, nt], BF16, name="qT")
            kT = pp.tile([128, nt], BF16, name="kT")
            # v in fp8 with an 80-byte chunk stride (DoubleRow k-tile dim
            # step must be a multiple of 16); cols 68-79 memset to zero.
            vaug = pp.tile([128, nkc, 80], FP8, name="vaug")
            # out-proj stationary per block, double-buffered; rows 68-127
            # must read zero in the proj matmul, so memset once and only
            # ever write rows 0..67.
            oT = [pp.tile([128, 512], BF16, name=f"oT{i}") for i in range(2)]
            nc.gpsimd.memset(warm[:], 0)
            nc.gpsimd.memset(vaug[:], 0)
            nc.gpsimd.memset(oT[0][:], 0)
            nc.gpsimd.memset(oT[1][:], 0)

            # ---- deferred-work FIFO: AV pairs, oT casts, projections ----
            o_ps_tiles = {}
            ysb_tiles = {}
            avq = []

            def pop_work(budget, floor=0):
                spent = 0
                while len(avq) > floor and spent < budget:
                    item = avq[0]
                    kind = item[0]
                    if kind == "av":
                        _, b, E, loc, j = item
                        if b not in o_ps_tiles:
                            o_ps_tiles[b] = psO.tile(
                                [80, 512], F32, tag="o", bufs=2, name="o_ps"
                            )
                        o_ps = o_ps_tiles[b]
                        nc.tensor.matmul(
                            o_ps[:],
                            vaug[:, 2 * j : 2 * j + 2, :],
                            E[:, 2 * loc : 2 * loc + 2, :],
                            start=(j == 0),
                            stop=(j == npair - 1),
                            perf_mode=DR,
                            skip_group_check=True,
                        )
                        spent += 1
                    elif kind == "cast":
                        b = item[1]
                        nc.vector.tensor_copy(
                            oT[b % 2][0 : HD + 2, :], o_ps_tiles[b][0 : HD + 2, :]
                        )
                    else:  # proj piece: one 128-token slice of block b
                        _, b, t = item
                        oTt = oT[b % 2]
                        pt = psS.tile([128, EOUT], F32, tag="sc", bufs=3, name="pt")
                        st = oTt[:, t * 128 : (t + 1) * 128]
                        nc.tensor.matmul(
                            pt[:, :512], st, wp[:, :512], start=True, stop=True
                        )
                        nc.tensor.matmul(
                            pt[:, 512:EOUT],
                            st,
                            wp[:, 512:EOUT],
                            start=True,
                            stop=True,
                        )
                        if t == 0:
                            ysb_tiles[b] = yp.tile(
                                [128, 4, EOUT], BF16, tag="ysb", bufs=2, name="ysb"
                            )
                        nc.vector.tensor_copy(ysb_tiles[b][:, t, :], pt[:])
                        if t == 3:
                            # whole block rides out in one contiguous DMA
                            # (dram is [block][partition][piece][col]; host
                            # untangles the piece/partition order)
                            nc.sync.dma_start(y_d[b], ysb_tiles[b][:])
                        spent += 1
                    avq.pop(0)

            def push_block_done(b):
                avq.append(("cast", b))
                for t in range(4):
                    avq.append(("proj", b, t))

            def emit_pair(b, j):
                qs = slice(b * 512, (b + 1) * 512)
                sc = psS.tile([128, 2, 512], F32, tag="sc", bufs=3, name="sc")
                for i in range(2):
                    kc = 2 * j + i
                    nc.tensor.matmul(
                        sc[:, i, :],
                        kT[:, kc * 128 : (kc + 1) * 128],
                        qT[:, qs],
                        start=True,
                        stop=True,
                    )
                E = ep.tile([128, 2, 512], FP8, tag="E", bufs=16, name="E")
                use_dve = j in (DVE_PAIRS_B0 if b == 0 else DVE_PAIRS)
                if use_dve:
                    nc.vector.tensor_scalar(
                        E[:].bitcast(I8), sc[:], A_SCH, B_SCH, MULT, ADD
                    )
                else:
                    nc.scalar.activation(E[:], sc[:], AF.Exp)
                avq.append(("av", b, E, 0, j))

            # ---------------- Phase A + block-0 scores ----------------
            with tc.tile_pool(name="psA", bufs=1, space="PSUM") as psA:
                # PE p-state warmup through the sca ring while x DMA lands
                wps = psS.tile([128, 2, 512], F32, tag="sc", bufs=3, name="sc")
                for _ in range(24):
                    nc.tensor.matmul(
                        wps[0:16, 0, 0:16], warm[:], warm[:], start=True, stop=True
                    )

                for b in range(nqb):
                    qs = slice(b * 512, (b + 1) * 512)
                    ps_q = psA.tile([128, 512], F32, tag="qk", bufs=2, name="ps_q")
                    ps_k = psA.tile([128, 512], F32, tag="qk", bufs=2, name="ps_k")
                    for c in range(NCH):
                        for w, ps in ((wq, ps_q), (wk, ps_k)):
                            nc.tensor.matmul(
                                ps[:],
                                w[:, c, :],
                                xT[:, c, qs],
                                start=(c == 0),
                                stop=(c == NCH - 1),
                            )
                    nc.scalar.activation(qT[:, qs], ps_q[:], AF.Copy)
                    nc.vector.tensor_copy(kT[:, qs], ps_k[:])
                    # block 0's score pairs over the k-chunks this qk block
                    # just produced; exp starts ~9us into the kernel
                    emit_pair(0, 2 * b)
                    emit_pair(0, 2 * b + 1)
                # block 0's deferred work queues ahead of block 1's early
                # AVs so cast(0) releases the psO ring before they pop
                push_block_done(0)
                # v: two token-block chains in flight; six of block 1's
                # score pairs interleave so ScalarE/DVE keep working
                # through the v-pass
                early_b1 = {6: 0, 10: 1, 14: 2, 18: 3, 22: 4, 26: 5}
                for t0 in range(0, nkc, 2):
                    psvs = [
                        psA.tile([128, HD + 2], F32, tag="qk", bufs=2, name="ps_v")
                        for _ in range(2)
                    ]
                    for c in range(NCH):
                        for i in range(2):
                            ts_ = slice((t0 + i) * 128, (t0 + i + 1) * 128)
                            nc.tensor.matmul(
                                psvs[i][:],
                                xT[:, c, ts_],
                                wv[:, c, :],
                                start=(c == 0),
                                stop=(c == NCH - 1),
                            )
                    for i in range(2):
                        nc.vector.tensor_copy(
                            vaug[:, t0 + i, 0 : HD + 2], psvs[i][:]
                        )
                    if t0 in early_b1:
                        emit_pair(1, early_b1[t0])

            # ------------- Phase B: blocks 1-7 + deferred work -------------
            with tc.tile_pool(name="psO", bufs=1, space="PSUM") as psO:
                for b in range(1, nqb):
                    last = b == nqb - 1
                    for j in range(npair):
                        if b == 1 and j < 6:
                            continue  # emitted during the v-pass
                        emit_pair(b, j)
                        pop_work(3 if last else 2, floor=0 if last else 2)
                    push_block_done(b)
                # drain
                pop_work(10**9)

    nc.compile()
    return nc


def _prep_inputs(x, w_qkv, b_qkv, w_proj, nt):
    """Host-side shard prep: returns list of 8 in_maps."""
    x = np.asarray(x, dtype=np.float32)
    w_qkv = np.asarray(w_qkv, dtype=np.float32)
    b_qkv = np.asarray(b_qkv, dtype=np.float32)
    w_proj = np.asarray(w_proj, dtype=np.float32)

    xt = x.reshape(nt, EMBED)
    xT_pad = np.zeros((NCH * 128, nt), dtype=np.float32)
    xT_pad[:EMBED] = xt.T
    xT_pad[EMBED] = 1.0
    # [128, NCH, nt]: partition-major to match the SBUF tile layout
    xT_in = np.ascontiguousarray(
        xT_pad.reshape(NCH, 128, nt).transpose(1, 0, 2)
    ).astype(BF16_NP)

    s = float(HD) ** -0.5
    in_maps = []
    for h in range(NHEADS):
        sl_q = slice(h * HD, (h + 1) * HD)
        sl_k = slice(EMBED + h * HD, EMBED + (h + 1) * HD)
        sl_v = slice(2 * EMBED + h * HD, 2 * EMBED + (h + 1) * HD)

        wq_t = np.zeros((NCH * 128, 128), dtype=np.float32)
        wq_t[:EMBED, :HD] = (w_qkv[sl_q] * s).T
        wq_t[EMBED, :HD] = b_qkv[sl_q] * s

        wk_t = np.zeros((NCH * 128, 128), dtype=np.float32)
        wk_t[:EMBED, :HD] = w_qkv[sl_k].T
        wk_t[EMBED, :HD] = b_qkv[sl_k]

        # ones column at index 0 so the softmax denominator lands on
        # PSUM partition 0 (-> oT row 0)
        wv_t = np.zeros((NCH * 128, HD + 2), dtype=np.float32)
        wv_t[:EMBED, 1 : HD + 1] = w_qkv[sl_v].T
        wv_t[EMBED, 1 : HD + 1] = b_qkv[sl_v]
        wv_t[EMBED, 0] = 1.0

        # proj weights: row 0 = denom row: zero into data cols, 1.0 into
        # col 528 so y[:, 528] = softmax denominator per token
        wp_t = np.zeros((128, EOUT), dtype=np.float32)
        wp_t[1 : HD + 1, :EMBED] = w_proj[:, sl_q].T
        wp_t[0, EMBED] = 1.0

        in_maps.append(
            {
                "xT": xT_in,
                "wq": np.ascontiguousarray(
                    wq_t.reshape(NCH, 128, 128).transpose(1, 0, 2)
                ).astype(BF16_NP),
                "wk": np.ascontiguousarray(
                    wk_t.reshape(NCH, 128, 128).transpose(1, 0, 2)
                ).astype(BF16_NP),
                "wv": np.ascontiguousarray(
                    wv_t.reshape(NCH, 128, HD + 2).transpose(1, 0, 2)
                ).astype(BF16_NP),
                "wp": wp_t.astype(BF16_NP),
            }
        )
    return in_maps


_NC_CACHE = {}


def _get_nc(nt=NT):
    if nt not in _NC_CACHE:
        _NC_CACHE[nt] = _build_nc(nt)
    return _NC_CACHE[nt]


def kernel(x, w_qkv, b_qkv, w_proj, b_proj, _trace=False):
    from concourse.bass_utils import run_bass_kernel_spmd

    x = np.asarray(x, dtype=np.float32)
    b_proj = np.asarray(b_proj, dtype=np.float32)
    B, D, H, W, C = x.shape
    nt = D * H * W

    nc = _get_nc(nt)
    in_maps = _prep_inputs(x, w_qkv, b_qkv, w_proj, nt)
    res = run_bass_kernel_spmd(
        nc, in_maps, core_ids=list(range(NHEADS)), trace=_trace
    )
    out = np.zeros((nt, EMBED), dtype=np.float32)
    for r in res.results:
        yraw = np.asarray(r["y"], dtype=np.float32)
        # [block][partition][piece][col] -> row = block*512 + piece*128 + p
        yfull = yraw.transpose(0, 2, 1, 3).reshape(nt, EOUT)
        out += yfull[:, :EMBED] / yfull[:, EMBED : EMBED + 1]
    out += b_proj
    kernel.last_results = res
    return out.reshape(B, D, H, W, C)
